# revision 37
# baseline (speedup 1.0000x reference)
"""Tensor-parallel GQA attention block (dense_transformer) on 8 TRN2 NeuronCores.

Sharding: tensor parallel across heads — core c owns q-heads 4c..4c+3 and
kv-head c (GQA groups intact). Instead of a row-parallel wo + AllReduce,
each core AllGathers the softmax-normalized per-head attention outputs y
(bf16, 2 pair-level collectives) and computes a 512-column slice of the
output projection; the host concatenates slices. This moves ~16x less data
than an fp32 AllReduce of the output.

Device-side design:
  - all matmul operands live "contraction dim on partitions": xT [DIM,S],
    wqkvT (fused q|k|v) [DIM,768], woT [DIM,512]; scores are computed
    transposed (S^T = K-tile^T @ Q^T) so no on-chip transposes of P are
    needed; V is PE-transposed once to [s,hd] for the PV matmul.
  - q/k head dims are permuted host-side so RoPE pairs sit 16 partitions
    apart inside one 32-partition quadrant: the rotation's half-swap is a
    single DVE stream_shuffle (dot products are invariant to the common
    permutation). Softmax scale is folded into wq.
  - softmax skips max-subtraction (scores are O(10); exp cannot overflow
    f32); masked logits get -1e9 from a sliceable causal band image, and
    fully-masked score halves are skipped entirely.
  - denominators: ones-vector matmuls accumulate row sums of exp(S^T);
    1/D via a fast custom-DVE reciprocal, broadcast across partitions on
    the (otherwise idle) GpSimd, applied off the PE critical path after a
    fast PSUM eviction copy.
  - software-pipelined emission per 512/1024-seq chunk keeps every
    engine's in-order stream free of cross-phase stalls:
        proj(0..2), attn(pair0)+AG0, proj(3), attn(pair1)+AG1,
        outproj(0..3)
    so both AllGathers hide under compute, PSUM banks (4 double-bank
    slots) hand over in <3us, and the PE never re-throttles its clock.
  - compute dtype: bf16 matmul operands (fp32 PSUM accumulation), fp32
    RoPE/softmax arithmetic. Measured rel err vs the fp32 reference:
    ~4e-3 (gate 2e-2).
"""

import ml_dtypes
import numpy as np

import concourse.bass as bass
import concourse.mybir as mybir
import concourse.tile as tile
from concourse import bacc
from concourse.bass_utils import run_bass_kernel_spmd

F32 = mybir.dt.float32
F32R = mybir.dt.float32r
BF16 = mybir.dt.bfloat16
AF = mybir.ActivationFunctionType

N_CORES = 8
DIM = 4096
S = 2048
HEAD_DIM = 128
N_HEADS = 32
N_KV = 8
HPC = N_HEADS // N_CORES        # q heads per core = 4
P = 128
SC = 512                        # seq chunk (free dim of most matmuls)
IC = 1024                       # attention i-chunk (2 seq chunks)
N_SCHUNK = S // SC              # 4
N_KTILE = DIM // P              # 32
N_STILE = S // P                # 16

SWAP16 = list(range(16, 32)) + list(range(16))   # per-quadrant 16-rotation


def build(debug_taps: bool = False):
    nc = bacc.Bacc(None, num_devices=N_CORES)

    xT = nc.declare_dram_parameter("xT", [DIM, S], BF16, isOutput=False)
    # fused qkv weights: [:, 0:512] q heads, [:, 512:640] k, [:, 640:768] v
    wqkvT = nc.declare_dram_parameter("wqkvT", [DIM, 768], BF16, isOutput=False)
    woT = nc.declare_dram_parameter("woT", [DIM, SC], BF16, isOutput=False)
    cosd = nc.declare_dram_parameter("cosd", [P, S], F32, isOutput=False)
    sins = nc.declare_dram_parameter("sins", [P, S], F32, isOutput=False)
    # causal band image: cmask[j, v] = 0 if (v - IC) >= j else -1e9
    cmask = nc.declare_dram_parameter("cmask", [P, 2 * IC], F32, isOutput=False)
    out = nc.dram_tensor("out", [S, SC], F32, kind="ExternalOutput")

    taps = {}
    if debug_taps:
        taps["qt"] = nc.dram_tensor("qt", [P, HPC, S], F32, kind="ExternalOutput")
        taps["kt"] = nc.dram_tensor("kt", [P, S], F32, kind="ExternalOutput")
        taps["vv"] = nc.dram_tensor("vv", [P, N_STILE, HEAD_DIM], F32, kind="ExternalOutput")
        taps["dd"] = nc.dram_tensor("dd", [HPC, S], F32, kind="ExternalOutput")
        taps["yl"] = nc.dram_tensor("yl", [P, HPC, S], BF16, kind="ExternalOutput")

    with tile.TileContext(nc) as tc:
        # PSUM: four 2-bank slots A-D.
        #   proj:    A={q0,q1} B={q2,q3} C={k,v} D=V-transpose scratch
        #   attn:    A/B = scores (2-deep, [P,1024]) C = psy D = psd
        #   outproj: C/D = pso (one [P,512] half each)
        ps = tc.alloc_tile_pool(name="ps", bufs=1, space="PSUM")
        const = tc.alloc_tile_pool(name="const", bufs=1)
        pw2 = tc.alloc_tile_pool(name="pw2", bufs=1, side="right")
        pw = tc.alloc_tile_pool(name="pw", bufs=1, side="right")
        main = tc.alloc_tile_pool(name="main", bufs=1)
        stream = tc.alloc_tile_pool(name="stream", bufs=3)
        tmp = tc.alloc_tile_pool(name="tmp", bufs=2)
        dram = tc.alloc_tile_pool(name="dram", bufs=1, space="DRAM")

        # ---- constants ---------------------------------------------------
        ones_f = const.tile([P, P], F32)
        nc.vector.memset(ones_f[:], 1.0)
        ones = const.tile([P, P], BF16)
        nc.scalar.copy(ones[:], ones_f[:])
        ident = const.tile([P, P], F32)
        from concourse.masks import make_identity
        make_identity(nc, ident[:])
        mask_sb = const.tile([P, 2 * IC], F32)
        nc.gpsimd.dma_start(mask_sb[:], cmask[:])
        cos_sb = pw.tile([P, S], F32)
        sin_sb = pw.tile([P, S], F32)
        nc.gpsimd.dma_start(cos_sb[:], cosd[:])
        nc.gpsimd.dma_start(sin_sb[:], sins[:])
        wqkv_sb = pw.tile([P, N_KTILE, 768], BF16)
        for k in range(N_KTILE):
            nc.scalar.dma_start(wqkv_sb[:, k, :], wqkvT[k * P:(k + 1) * P, :])
        wo_sb = pw2.tile([P, N_KTILE, SC], BF16)

        kt_sb = main.tile([P, S], BF16)
        v_sb = main.tile([P, N_STILE, HEAD_DIM], BF16)
        qt_sb = main.tile([P, HPC, S], BF16)

        ybounce = [
            dram.tile([HPC * P, IC], BF16, name=f"ybounce{cp}")
            for cp in range(2)
        ]
        ygather = [
            dram.tile([N_CORES * HPC * P, IC], BF16, addr_space="Shared",
                      name=f"ygather{cp}")
            for cp in range(2)
        ]
        qt_tiles = {}
        yg_tiles = {}

        # ---- per-chunk segment emitters ----------------------------------
        def proj(ci):
            s_lo = ci * SC
            cp, half = ci // 2, ci % 2
            psq01 = ps.tile([P, 2, SC], F32, tag="A", name=f"psq01_{ci}")
            psq23 = ps.tile([P, 2, SC], F32, tag="B", name=f"psq23_{ci}")
            pskv = ps.tile([P, 2, SC], F32, tag="C", name=f"pskv_{ci}")
            for k in range(N_KTILE):
                xs = stream.tile([P, SC], BF16, tag="xs", bufs=10, name=f"xs{ci}_{k}")
                nc.sync.dma_start(xs[:], xT[k * P:(k + 1) * P, s_lo:s_lo + SC])
                st = dict(start=(k == 0), stop=(k == N_KTILE - 1))
                for h in range(HPC):
                    dst = (psq01 if h < 2 else psq23)[:, h % 2, :]
                    nc.tensor.matmul(
                        dst, wqkv_sb[:, k, h * P:(h + 1) * P], xs[:], **st
                    )
                nc.tensor.matmul(pskv[:, 0, :], wqkv_sb[:, k, 512:640], xs[:], **st)
                nc.tensor.matmul(pskv[:, 1, :], wqkv_sb[:, k, 640:768], xs[:], **st)

            # evict all psum slices with single fast copies first (frees the
            # banks in ~2 DVE ops so the next chunk's matmuls keep PE warm),
            # then do the rope math from the SBUF copies.
            raws = []
            for h in range(HPC + 1):
                src = pskv[:, 0, :] if h == HPC \
                    else (psq01 if h < 2 else psq23)[:, h % 2, :]
                raw = tmp.tile([P, SC], F32, tag="rraw", bufs=6, name=f"rr{ci}_{h}")
                nc.vector.tensor_copy(raw[:], src)
                raws.append(raw)
            vt = tmp.tile([P, SC], F32, tag="vt", bufs=1, name=f"vt{ci}")
            nc.scalar.copy(vt[:], pskv[:, 1, :])

            for h in range(HPC + 1):
                raw = raws[h]
                dst = kt_sb[:, s_lo:s_lo + SC] if h == HPC \
                    else qt_sb[:, h, s_lo:s_lo + SC]
                qc = tmp.tile([P, SC], F32, tag="rqc", bufs=1, name=f"rq{ci}_{h}")
                nc.vector.tensor_mul(qc[:], raw[:], cos_sb[:, s_lo:s_lo + SC])
                qsw = tmp.tile([P, SC], F32, tag="rqs", bufs=1, name=f"rs{ci}_{h}")
                nc.vector.stream_shuffle(qsw[:], raw[:], SWAP16)
                nc.vector.tensor_mul(qsw[:], qsw[:], sin_sb[:, s_lo:s_lo + SC])
                nc.vector.tensor_add(dst, qc[:], qsw[:])

            pstT = ps.tile([P, 2, SC], F32, tag="D", name=f"pst{ci}")
            for q in range(SC // P):
                nc.tensor.transpose(
                    pstT[:, q % 2, 0:P], vt[:, q * P:(q + 1) * P], ident[:]
                )
                nc.vector.tensor_copy(v_sb[:, ci * 4 + q, :], pstT[:, q % 2, 0:P])

        def attn(cp):
            n_j = 8 * (cp + 1)
            for h in range(HPC):
                psy = ps.tile([P, IC], F32, tag="C", name=f"psy{h}_{cp}")
                psd = ps.tile([1, IC], F32, tag="D", name=f"psd{h}_{cp}")
                for t in range(n_j):
                    # u-half is computed only while not fully masked:
                    # half u covers i in [cp*IC + u*SC, +SC); j-tile t is
                    # fully masked for that half iff 128t >= cp*IC + u*SC.
                    u_list = [u for u in (0, 1) if t < 8 * cp + 4 + 4 * u]
                    pss = ps.tile(
                        [P, IC], F32, tag=("A" if t % 2 == 0 else "B"),
                        name=f"pss{h}_{cp}_{t}",
                    )
                    for u in u_list:
                        nc.tensor.matmul(
                            pss[:, u * SC:(u + 1) * SC],
                            kt_sb[:, t * P:(t + 1) * P],
                            qt_sb[:, h, cp * IC + u * SC:cp * IC + (u + 1) * SC],
                            start=True, stop=True,
                        )
                    pt = tmp.tile([P, IC], BF16, tag="pt", bufs=6,
                                  name=f"pt{h}_{cp}_{t}")
                    d = t - 8 * cp
                    if d < 0:
                        nc.scalar.activation(pt[:], pss[:], AF.Exp)
                    else:
                        for u in u_list:
                            sl = slice(u * SC, (u + 1) * SC)
                            if t <= 8 * cp + 4 * u - 1:
                                nc.scalar.activation(pt[:, sl], pss[:, sl], AF.Exp)
                            else:
                                ms = tmp.tile([P, SC], F32, tag="ms", bufs=3,
                                              name=f"ms{h}_{cp}_{t}_{u}")
                                nc.vector.tensor_add(
                                    ms[:], pss[:, sl],
                                    mask_sb[:, IC - P * d + u * SC:
                                            2 * IC - P * d + (u - 1) * SC],
                                )
                                nc.scalar.activation(pt[:, sl], ms[:], AF.Exp)
                    for u in u_list:
                        sl = slice(u * SC, (u + 1) * SC)
                        st = dict(start=(t == 0), stop=(t == 8 * cp + 3 + 4 * u))
                        nc.tensor.matmul(psy[:, sl], v_sb[:, t, :], pt[:, sl], **st)
                        nc.tensor.matmul(psd[:, sl], ones[:, 0:1], pt[:, sl], **st)

                # fast-evict psy (frees the C banks in one ACT op), then
                # normalize off the PE from the SBUF copy
                ysb = tmp.tile([P, IC], F32, tag="ysb", name=f"ysb{h}_{cp}")
                nc.vector.tensor_copy(ysb[:], psy[:])
                dsb = tmp.tile([1, IC], F32, tag="dsb", bufs=1, name=f"dsb{h}_{cp}")
                nc.vector.tensor_copy(dsb[:], psd[:])
                rc1 = tmp.tile([1, IC], F32, tag="rc1", bufs=1, name=f"rc1{h}_{cp}")
                nc.vector.reciprocal_approx_fast(rc1[:], dsb[:])
                rbb = tmp.tile([P, IC], F32, tag="rbb", bufs=1, name=f"rbb{h}_{cp}")
                nc.gpsimd.partition_broadcast(rbb[:], rc1[:])
                yp = tmp.tile([P, IC], BF16, tag="yp", name=f"yp{h}_{cp}")
                nc.vector.tensor_mul(yp[:], ysb[:], rbb[:])
                nc.gpsimd.dma_start(
                    ybounce[cp][h * P:(h + 1) * P, :], yp[:]
                )
                if debug_taps:
                    s_lo = cp * IC
                    nc.sync.dma_start(taps["yl"][:, h, s_lo:s_lo + IC], yp[:])
                    nc.sync.dma_start(taps["dd"][h:h + 1, s_lo:s_lo + IC], dsb[:])

            nc.gpsimd.collective_compute(
                "AllGather",
                mybir.AluOpType.bypass,
                replica_groups=[list(range(N_CORES))],
                ins=[ybounce[cp][:]],
                outs=[ygather[cp][:]],
            )

        def outproj(ci):
            g_lo = ci * SC
            cp, u = ci // 2, ci % 2
            if u == 0:
                yg_tiles[cp] = [
                    pyg.tile([P, 8, IC], BF16, tag=f"yg{q}",
                             name=f"yg{cp}_{q}")
                    for q in range(4)
                ]
                for q in range(4):
                    nc.sync.dma_start(
                        yg_tiles[cp][q][:],
                        ygather[cp][q * 8 * P:(q + 1) * 8 * P, :]
                        .rearrange("(t p) m -> p t m", p=P),
                    )
            for st_i in range(4):
                pso = ps.tile(
                    [P, SC], F32, tag=("C" if st_i % 2 == 0 else "D"),
                    name=f"pso{ci}_{st_i}",
                )
                for kt in range(N_KTILE):
                    src_t = yg_tiles[cp][kt // 8]
                    nc.tensor.matmul(
                        pso[:],
                        src_t[:, kt % 8, u * SC + st_i * P:u * SC + (st_i + 1) * P],
                        wo_sb[:, kt, :],
                        start=(kt == 0), stop=(kt == N_KTILE - 1),
                    )
                ob = tmp.tile([P, SC], F32, tag="ob", name=f"ob{ci}_{st_i}")
                nc.scalar.copy(ob[:], pso[:])
                nc.gpsimd.dma_start(
                    out[g_lo + st_i * P:g_lo + (st_i + 1) * P, :], ob[:]
                )

        # ---- software-pipelined emission ---------------------------------
        # attn(0) sits between proj(2)/proj(3) so AG_p0 hides under
        # proj(3)+attn(1); AG_p1 hides under outproj(0)/(1).
        proj(0)
        proj(1)
        proj(2)
        attn(0)          # -> AG pair 0
        nc.scalar.dma_start(wo_sb[:], woT.rearrange("(t p) m -> p t m", p=P))
        proj(3)
        attn(1)          # -> AG pair 1
        pw.release()
        pyg = tc.alloc_tile_pool(name="pyg", bufs=1, side="right")
        outproj(0)
        outproj(1)
        outproj(2)
        outproj(3)

        if debug_taps:
            nc.sync.dma_start(taps["qt"][:], qt_sb[:])
            nc.sync.dma_start(taps["kt"][:], kt_sb[:])
            nc.sync.dma_start(taps["vv"][:], v_sb[:])

        for pool in (pyg, pw2, dram, tmp, stream, main, const, ps):
            pool.release()

    nc.compile()
    return nc


# ---------------------------------------------------------------------------
# host-side prep / unshard
# ---------------------------------------------------------------------------

def _perm128():
    """head-dim permutation: pair i=(16q+j) -> even at 32q+j, odd at 32q+16+j."""
    order = np.empty(128, dtype=np.int64)
    for i in range(64):
        q, j = i // 16, i % 16
        order[32 * q + j] = 2 * i
        order[32 * q + 16 + j] = 2 * i + 1
    return order


def _host_prep(x, freqs_cis, wq, wk, wv, wo):
    order = _perm128()
    xT = np.ascontiguousarray(x[0].T)                       # [DIM, S]
    scale = np.float32(1.0 / np.sqrt(HEAD_DIM))

    cosT = np.ascontiguousarray(freqs_cis[:, :, 0].T)       # [64, S]
    sinT = np.ascontiguousarray(freqs_cis[:, :, 1].T)
    cosd = np.empty((P, S), dtype=np.float32)
    sins = np.empty((P, S), dtype=np.float32)
    for q in range(4):
        cosd[32 * q:32 * q + 16] = cosT[16 * q:16 * q + 16]
        cosd[32 * q + 16:32 * q + 32] = cosT[16 * q:16 * q + 16]
        sins[32 * q:32 * q + 16] = -sinT[16 * q:16 * q + 16]
        sins[32 * q + 16:32 * q + 32] = sinT[16 * q:16 * q + 16]

    vv = np.arange(2 * IC)[None, :]
    jj = np.arange(P)[:, None]
    cmask = np.where(vv - IC >= jj, np.float32(0.0), np.float32(-1e9))
    cmask = np.ascontiguousarray(cmask, dtype=np.float32)

    xT16 = xT.astype(ml_dtypes.bfloat16)
    in_maps = []
    for c in range(N_CORES):
        wq_c = wq[c * 512:(c + 1) * 512].reshape(HPC, 128, DIM)[:, order, :]
        wq_c = (wq_c.reshape(512, DIM) * scale).astype(np.float32)
        wk_c = wk[c * 128:(c + 1) * 128][order]
        wv_c = wv[c * 128:(c + 1) * 128]
        wqkv_c = np.concatenate([wq_c, wk_c, wv_c], axis=0)
        wo_c = wo[c * 512:(c + 1) * 512]
        in_maps.append({
            "xT": xT16,
            "wqkvT": np.ascontiguousarray(wqkv_c.T).astype(ml_dtypes.bfloat16),
            "woT": np.ascontiguousarray(wo_c.T).astype(ml_dtypes.bfloat16),
            "cosd": cosd,
            "sins": sins,
            "cmask": cmask,
        })
    return in_maps


_NC_CACHE = {}


def get_nc(debug_taps=False):
    key = bool(debug_taps)
    if key not in _NC_CACHE:
        _NC_CACHE[key] = build(debug_taps=key)
    return _NC_CACHE[key]


def kernel(x, freqs_cis, mask, wq, wk, wv, wo, _trace=False, _debug_taps=False,
           _warmup=False):
    in_maps = _host_prep(x, freqs_cis, wq, wk, wv, wo)
    nc = get_nc(_debug_taps)
    if _warmup:
        run_bass_kernel_spmd(
            nc, in_maps, core_ids=list(range(N_CORES)), trace=False
        )
    res = run_bass_kernel_spmd(
        nc, in_maps, core_ids=list(range(N_CORES)), trace=_trace
    )
    full = np.concatenate([res.results[c]["out"] for c in range(N_CORES)], axis=1)
    out = full.reshape(1, S, DIM).astype(np.float32)
    if _trace or _debug_taps:
        kernel.last_results = res
    return out


# revision 38
# speedup vs baseline: 1.1082x; 1.1082x over previous
"""Tensor-parallel GQA attention block (dense_transformer) on 8 TRN2 NeuronCores.

Sharding: tensor parallel across heads — core c owns q-heads 4c..4c+3 and
kv-head c (GQA groups intact). Instead of a row-parallel wo + AllReduce,
each core AllGathers the softmax-normalized per-head attention outputs y
(bf16, 2 pair-level collectives) and computes a 512-column slice of the
output projection; the host concatenates slices. This moves ~16x less data
than an fp32 AllReduce of the output.

Device-side design:
  - all matmul operands live "contraction dim on partitions": xT [DIM,S],
    wqkvT (fused q|k|v) [DIM,768], woT [DIM,512]; scores are computed
    transposed (S^T = K-tile^T @ Q^T) so no on-chip transposes of P are
    needed; V is PE-transposed once to [s,hd] for the PV matmul.
  - q/k head dims are permuted host-side so RoPE pairs sit 16 partitions
    apart inside one 32-partition quadrant: the rotation's half-swap is a
    single DVE stream_shuffle (dot products are invariant to the common
    permutation). Softmax scale is folded into wq.
  - softmax skips max-subtraction (scores are O(10); exp cannot overflow
    f32); masked logits get -1e9 from a sliceable causal band image, and
    fully-masked score halves are skipped entirely.
  - denominators: ones-vector matmuls accumulate row sums of exp(S^T);
    1/D via a fast custom-DVE reciprocal, broadcast across partitions on
    the (otherwise idle) GpSimd, applied off the PE critical path after a
    fast PSUM eviction copy.
  - software-pipelined emission per 512/1024-seq chunk keeps every
    engine's in-order stream free of cross-phase stalls:
        proj(0..2), attn(pair0)+AG0, proj(3), attn(pair1)+AG1,
        outproj(0..3)
    so both AllGathers hide under compute, PSUM banks (4 double-bank
    slots) hand over in <3us, and the PE never re-throttles its clock.
  - compute dtype: bf16 matmul operands (fp32 PSUM accumulation), fp32
    RoPE/softmax arithmetic. Measured rel err vs the fp32 reference:
    ~4e-3 (gate 2e-2).
"""

import ml_dtypes
import numpy as np

import concourse.bass as bass
import concourse.mybir as mybir
import concourse.tile as tile
from concourse import bacc
from concourse.bass_utils import run_bass_kernel_spmd

F32 = mybir.dt.float32
F32R = mybir.dt.float32r
BF16 = mybir.dt.bfloat16
AF = mybir.ActivationFunctionType

N_CORES = 8
DIM = 4096
S = 2048
HEAD_DIM = 128
N_HEADS = 32
N_KV = 8
HPC = N_HEADS // N_CORES        # q heads per core = 4
P = 128
SC = 512                        # seq chunk (free dim of most matmuls)
IC = 1024                       # attention i-chunk (2 seq chunks)
N_SCHUNK = S // SC              # 4
N_KTILE = DIM // P              # 32
N_STILE = S // P                # 16

SWAP16 = list(range(16, 32)) + list(range(16))   # per-quadrant 16-rotation


def build(debug_taps: bool = False):
    nc = bacc.Bacc(None, num_devices=N_CORES)

    xT = nc.declare_dram_parameter("xT", [DIM, S], BF16, isOutput=False)
    # fused qkv weights: [:, 0:512] q heads, [:, 512:640] k, [:, 640:768] v
    wqkvT = nc.declare_dram_parameter("wqkvT", [DIM, 768], BF16, isOutput=False)
    woT = nc.declare_dram_parameter("woT", [DIM, SC], BF16, isOutput=False)
    cosd = nc.declare_dram_parameter("cosd", [P, S], F32, isOutput=False)
    sins = nc.declare_dram_parameter("sins", [P, S], F32, isOutput=False)
    # causal band image: cmask[j, v] = 0 if (v - IC) >= j else -1e9
    cmask = nc.declare_dram_parameter("cmask", [P, 2 * IC], F32, isOutput=False)
    out = nc.dram_tensor("out", [S, SC], F32, kind="ExternalOutput")

    taps = {}
    if debug_taps:
        taps["qt"] = nc.dram_tensor("qt", [P, HPC, S], F32, kind="ExternalOutput")
        taps["kt"] = nc.dram_tensor("kt", [P, S], F32, kind="ExternalOutput")
        taps["vv"] = nc.dram_tensor("vv", [P, N_STILE, HEAD_DIM], F32, kind="ExternalOutput")
        taps["dd"] = nc.dram_tensor("dd", [HPC, S], F32, kind="ExternalOutput")
        taps["yl"] = nc.dram_tensor("yl", [P, HPC, S], BF16, kind="ExternalOutput")

    with tile.TileContext(nc) as tc:
        # PSUM: four 2-bank slots A-D.
        #   proj:    A={q0,q1} B={q2,q3} C={k,v} D=V-transpose scratch
        #   attn:    A/B = scores (2-deep, [P,1024]) C = psy D = psd
        #   outproj: C/D = pso (one [P,512] half each)
        ps = tc.alloc_tile_pool(name="ps", bufs=1, space="PSUM")
        const = tc.alloc_tile_pool(name="const", bufs=1)
        pw2 = tc.alloc_tile_pool(name="pw2", bufs=1, side="right")
        pw = tc.alloc_tile_pool(name="pw", bufs=1, side="right")
        main = tc.alloc_tile_pool(name="main", bufs=1)
        stream = tc.alloc_tile_pool(name="stream", bufs=3)
        tmp = tc.alloc_tile_pool(name="tmp", bufs=2)
        dram = tc.alloc_tile_pool(name="dram", bufs=1, space="DRAM")

        # ---- constants ---------------------------------------------------
        ones_f = const.tile([P, P], F32)
        nc.vector.memset(ones_f[:], 1.0)
        ones = const.tile([P, P], BF16)
        nc.scalar.copy(ones[:], ones_f[:])
        ident = const.tile([P, P], F32)
        from concourse.masks import make_identity
        make_identity(nc, ident[:])
        mask_sb = const.tile([P, 2 * IC], F32)
        nc.gpsimd.dma_start(mask_sb[:], cmask[:])
        cos_sb = pw.tile([P, S], F32)
        sin_sb = pw.tile([P, S], F32)
        nc.gpsimd.dma_start(cos_sb[:], cosd[:])
        nc.gpsimd.dma_start(sin_sb[:], sins[:])
        wqkv_sb = pw.tile([P, N_KTILE, 768], BF16)
        for k in range(N_KTILE):
            nc.scalar.dma_start(wqkv_sb[:, k, :], wqkvT[k * P:(k + 1) * P, :])
        wo_sb = pw2.tile([P, N_KTILE, SC], BF16)

        kt_sb = main.tile([P, S], BF16)
        v_sb = main.tile([P, N_STILE, HEAD_DIM], BF16)
        qt_sb = main.tile([P, HPC, S], BF16)

        ybounce = [
            dram.tile([HPC * P, IC], BF16, name=f"ybounce{cp}")
            for cp in range(2)
        ]
        ygather = [
            dram.tile([N_CORES * HPC * P, IC], BF16, addr_space="Shared",
                      name=f"ygather{cp}")
            for cp in range(2)
        ]
        qt_tiles = {}
        yg_tiles = {}

        # ---- per-chunk segment emitters ----------------------------------
        def proj(ci):
            s_lo = ci * SC
            cp, half = ci // 2, ci % 2
            psq01 = ps.tile([P, 2, SC], F32, tag="A", name=f"psq01_{ci}")
            psq23 = ps.tile([P, 2, SC], F32, tag="B", name=f"psq23_{ci}")
            pskv = ps.tile([P, 2, SC], F32, tag="C", name=f"pskv_{ci}")
            for k in range(N_KTILE):
                xs = stream.tile([P, SC], BF16, tag="xs", bufs=12, name=f"xs{ci}_{k}")
                nc.sync.dma_start(xs[:], xT[k * P:(k + 1) * P, s_lo:s_lo + SC])
                st = dict(start=(k == 0), stop=(k == N_KTILE - 1))
                for h in range(HPC):
                    dst = (psq01 if h < 2 else psq23)[:, h % 2, :]
                    nc.tensor.matmul(
                        dst, wqkv_sb[:, k, h * P:(h + 1) * P], xs[:], **st
                    )
                nc.tensor.matmul(pskv[:, 0, :], wqkv_sb[:, k, 512:640], xs[:], **st)
                nc.tensor.matmul(pskv[:, 1, :], wqkv_sb[:, k, 640:768], xs[:], **st)

            # evict all psum slices with single fast copies first (frees the
            # banks in ~2 DVE ops so the next chunk's matmuls keep PE warm),
            # then do the rope math from the SBUF copies.
            raws = []
            for h in range(HPC + 1):
                src = pskv[:, 0, :] if h == HPC \
                    else (psq01 if h < 2 else psq23)[:, h % 2, :]
                raw = tmp.tile([P, SC], F32, tag="rraw", bufs=6, name=f"rr{ci}_{h}")
                nc.scalar.copy(raw[:], src)
                raws.append(raw)
            vt = tmp.tile([P, SC], F32, tag="vt", bufs=1, name=f"vt{ci}")
            nc.scalar.copy(vt[:], pskv[:, 1, :])

            for h in range(HPC + 1):
                raw = raws[h]
                dst = kt_sb[:, s_lo:s_lo + SC] if h == HPC \
                    else qt_sb[:, h, s_lo:s_lo + SC]
                qc = tmp.tile([P, SC], F32, tag="rqc", bufs=1, name=f"rq{ci}_{h}")
                nc.vector.tensor_mul(qc[:], raw[:], cos_sb[:, s_lo:s_lo + SC])
                qsw = tmp.tile([P, SC], F32, tag="rqs", bufs=1, name=f"rs{ci}_{h}")
                nc.vector.stream_shuffle(qsw[:], raw[:], SWAP16)
                nc.vector.tensor_mul(qsw[:], qsw[:], sin_sb[:, s_lo:s_lo + SC])
                nc.vector.tensor_add(dst, qc[:], qsw[:])

            pstT = ps.tile([P, 2, SC], F32, tag="D", name=f"pst{ci}")
            for q in range(SC // P):
                nc.tensor.transpose(
                    pstT[:, q % 2, 0:P], vt[:, q * P:(q + 1) * P], ident[:]
                )
                nc.vector.tensor_copy(v_sb[:, ci * 4 + q, :], pstT[:, q % 2, 0:P])

        def attn(cp):
            n_j = 8 * (cp + 1)
            for h in range(HPC):
                psy = ps.tile([P, IC], F32, tag="C", name=f"psy{h}_{cp}")
                psd = ps.tile([1, IC], F32, tag="D", name=f"psd{h}_{cp}")
                for t in range(n_j):
                    # u-half is computed only while not fully masked:
                    # half u covers i in [cp*IC + u*SC, +SC); j-tile t is
                    # fully masked for that half iff 128t >= cp*IC + u*SC.
                    u_list = [u for u in (0, 1) if t < 8 * cp + 4 + 4 * u]
                    pss = ps.tile(
                        [P, IC], F32, tag=("A" if t % 2 == 0 else "B"),
                        name=f"pss{h}_{cp}_{t}",
                    )
                    for u in u_list:
                        nc.tensor.matmul(
                            pss[:, u * SC:(u + 1) * SC],
                            kt_sb[:, t * P:(t + 1) * P],
                            qt_sb[:, h, cp * IC + u * SC:cp * IC + (u + 1) * SC],
                            start=True, stop=True,
                        )
                    pt = tmp.tile([P, IC], BF16, tag="pt", bufs=6,
                                  name=f"pt{h}_{cp}_{t}")
                    d = t - 8 * cp
                    if d < 0:
                        nc.scalar.activation(pt[:], pss[:], AF.Exp)
                    else:
                        for u in u_list:
                            sl = slice(u * SC, (u + 1) * SC)
                            if t <= 8 * cp + 4 * u - 1:
                                nc.scalar.activation(pt[:, sl], pss[:, sl], AF.Exp)
                            else:
                                ms = tmp.tile([P, SC], F32, tag="ms", bufs=3,
                                              name=f"ms{h}_{cp}_{t}_{u}")
                                nc.vector.tensor_add(
                                    ms[:], pss[:, sl],
                                    mask_sb[:, IC - P * d + u * SC:
                                            2 * IC - P * d + (u - 1) * SC],
                                )
                                nc.scalar.activation(pt[:, sl], ms[:], AF.Exp)
                    for u in u_list:
                        sl = slice(u * SC, (u + 1) * SC)
                        st = dict(start=(t == 0), stop=(t == 8 * cp + 3 + 4 * u))
                        nc.tensor.matmul(psy[:, sl], v_sb[:, t, :], pt[:, sl], **st)
                        nc.tensor.matmul(psd[:, sl], ones[:, 0:1], pt[:, sl], **st)

                # fast-evict psy (frees the C banks in one ACT op), then
                # normalize off the PE from the SBUF copy
                ysb = tmp.tile([P, IC], F32, tag="ysb", name=f"ysb{h}_{cp}")
                nc.vector.tensor_copy(ysb[:], psy[:])
                dsb = tmp.tile([1, IC], F32, tag="dsb", bufs=1, name=f"dsb{h}_{cp}")
                nc.vector.tensor_copy(dsb[:], psd[:])
                rc1 = tmp.tile([1, IC], F32, tag="rc1", bufs=1, name=f"rc1{h}_{cp}")
                nc.vector.reciprocal_approx_fast(rc1[:], dsb[:])
                rbb = tmp.tile([P, IC], F32, tag="rbb", bufs=1, name=f"rbb{h}_{cp}")
                nc.gpsimd.partition_broadcast(rbb[:], rc1[:])
                yp = tmp.tile([P, IC], BF16, tag="yp", name=f"yp{h}_{cp}")
                nc.vector.tensor_mul(yp[:], ysb[:], rbb[:])
                nc.gpsimd.dma_start(
                    ybounce[cp][h * P:(h + 1) * P, :], yp[:]
                )
                if debug_taps:
                    s_lo = cp * IC
                    nc.sync.dma_start(taps["yl"][:, h, s_lo:s_lo + IC], yp[:])
                    nc.sync.dma_start(taps["dd"][h:h + 1, s_lo:s_lo + IC], dsb[:])

            nc.gpsimd.collective_compute(
                "AllGather",
                mybir.AluOpType.bypass,
                replica_groups=[list(range(N_CORES))],
                ins=[ybounce[cp][:]],
                outs=[ygather[cp][:]],
            )

        def outproj(ci):
            g_lo = ci * SC
            cp, u = ci // 2, ci % 2
            if u == 0:
                yg_tiles[cp] = [
                    pyg.tile([P, 8, IC], BF16, tag=f"yg{q}",
                             name=f"yg{cp}_{q}")
                    for q in range(4)
                ]
                for q in range(4):
                    nc.sync.dma_start(
                        yg_tiles[cp][q][:],
                        ygather[cp][q * 8 * P:(q + 1) * 8 * P, :]
                        .rearrange("(t p) m -> p t m", p=P),
                    )
            for st_i in range(4):
                pso = ps.tile(
                    [P, SC], F32, tag=("C" if st_i % 2 == 0 else "D"),
                    name=f"pso{ci}_{st_i}",
                )
                for kt in range(N_KTILE):
                    src_t = yg_tiles[cp][kt // 8]
                    nc.tensor.matmul(
                        pso[:],
                        src_t[:, kt % 8, u * SC + st_i * P:u * SC + (st_i + 1) * P],
                        wo_sb[:, kt, :],
                        start=(kt == 0), stop=(kt == N_KTILE - 1),
                    )
                ob = tmp.tile([P, SC], F32, tag="ob", name=f"ob{ci}_{st_i}")
                nc.scalar.copy(ob[:], pso[:])
                nc.gpsimd.dma_start(
                    out[g_lo + st_i * P:g_lo + (st_i + 1) * P, :], ob[:]
                )

        # ---- software-pipelined emission ---------------------------------
        # attn(0) sits between proj(2)/proj(3) so AG_p0 hides under
        # proj(3)+attn(1); AG_p1 hides under outproj(0)/(1).
        proj(0)
        proj(1)
        proj(2)
        attn(0)          # -> AG pair 0
        nc.scalar.dma_start(wo_sb[:], woT.rearrange("(t p) m -> p t m", p=P))
        proj(3)
        attn(1)          # -> AG pair 1
        pw.release()
        pyg = tc.alloc_tile_pool(name="pyg", bufs=1, side="right")
        outproj(0)
        outproj(1)
        outproj(2)
        outproj(3)

        if debug_taps:
            nc.sync.dma_start(taps["qt"][:], qt_sb[:])
            nc.sync.dma_start(taps["kt"][:], kt_sb[:])
            nc.sync.dma_start(taps["vv"][:], v_sb[:])

        for pool in (pyg, pw2, dram, tmp, stream, main, const, ps):
            pool.release()

    nc.compile()
    return nc


# ---------------------------------------------------------------------------
# host-side prep / unshard
# ---------------------------------------------------------------------------

def _perm128():
    """head-dim permutation: pair i=(16q+j) -> even at 32q+j, odd at 32q+16+j."""
    order = np.empty(128, dtype=np.int64)
    for i in range(64):
        q, j = i // 16, i % 16
        order[32 * q + j] = 2 * i
        order[32 * q + 16 + j] = 2 * i + 1
    return order


def _host_prep(x, freqs_cis, wq, wk, wv, wo):
    order = _perm128()
    xT = np.ascontiguousarray(x[0].T)                       # [DIM, S]
    scale = np.float32(1.0 / np.sqrt(HEAD_DIM))

    cosT = np.ascontiguousarray(freqs_cis[:, :, 0].T)       # [64, S]
    sinT = np.ascontiguousarray(freqs_cis[:, :, 1].T)
    cosd = np.empty((P, S), dtype=np.float32)
    sins = np.empty((P, S), dtype=np.float32)
    for q in range(4):
        cosd[32 * q:32 * q + 16] = cosT[16 * q:16 * q + 16]
        cosd[32 * q + 16:32 * q + 32] = cosT[16 * q:16 * q + 16]
        sins[32 * q:32 * q + 16] = -sinT[16 * q:16 * q + 16]
        sins[32 * q + 16:32 * q + 32] = sinT[16 * q:16 * q + 16]

    vv = np.arange(2 * IC)[None, :]
    jj = np.arange(P)[:, None]
    cmask = np.where(vv - IC >= jj, np.float32(0.0), np.float32(-1e9))
    cmask = np.ascontiguousarray(cmask, dtype=np.float32)

    xT16 = xT.astype(ml_dtypes.bfloat16)
    in_maps = []
    for c in range(N_CORES):
        wq_c = wq[c * 512:(c + 1) * 512].reshape(HPC, 128, DIM)[:, order, :]
        wq_c = (wq_c.reshape(512, DIM) * scale).astype(np.float32)
        wk_c = wk[c * 128:(c + 1) * 128][order]
        wv_c = wv[c * 128:(c + 1) * 128]
        wqkv_c = np.concatenate([wq_c, wk_c, wv_c], axis=0)
        wo_c = wo[c * 512:(c + 1) * 512]
        in_maps.append({
            "xT": xT16,
            "wqkvT": np.ascontiguousarray(wqkv_c.T).astype(ml_dtypes.bfloat16),
            "woT": np.ascontiguousarray(wo_c.T).astype(ml_dtypes.bfloat16),
            "cosd": cosd,
            "sins": sins,
            "cmask": cmask,
        })
    return in_maps


_NC_CACHE = {}


def get_nc(debug_taps=False):
    key = bool(debug_taps)
    if key not in _NC_CACHE:
        _NC_CACHE[key] = build(debug_taps=key)
    return _NC_CACHE[key]


def kernel(x, freqs_cis, mask, wq, wk, wv, wo, _trace=False, _debug_taps=False,
           _warmup=False):
    in_maps = _host_prep(x, freqs_cis, wq, wk, wv, wo)
    nc = get_nc(_debug_taps)
    if _warmup:
        run_bass_kernel_spmd(
            nc, in_maps, core_ids=list(range(N_CORES)), trace=False
        )
    res = run_bass_kernel_spmd(
        nc, in_maps, core_ids=list(range(N_CORES)), trace=_trace
    )
    full = np.concatenate([res.results[c]["out"] for c in range(N_CORES)], axis=1)
    out = full.reshape(1, S, DIM).astype(np.float32)
    if _trace or _debug_taps:
        kernel.last_results = res
    return out
